# revision 18
# baseline (speedup 1.0000x reference)
"""CTConv2d Trainium2 kernel — fp8 DoubleRow edition.

Computes y = conv2d(x, w) where w (O,I,3,3) is synthesized on host from
core/periphery/threshold/scale, and the conv runs on 8 NeuronCores,
data-parallel over batch (32 images -> 4 per core).

Per 4-row output block the PE runs FIVE matmuls instead of the
baseline's seven:
  - 1 fp16 matmul for the dominant center tap (weights c), and
  - 4 fp8e4m3 DoubleRow matmuls, each computing TWO periphery taps at
    once: the PE's double-fp8 mode contracts two (weight, pixel) pairs
    per cell per cycle. The two taps of a pair read the SAME padded fp8
    image at two different (dh,dw) shifts via a 3D access pattern
    [partition][plane=2, stride=delta][pixel=464, stride 1] — zero data
    movement, no interleaving copies, and the Vector engine is entirely
    idle (the baseline spent ~115us of DVE time on tap pre-combining).

Periphery weights are tiny (|cg·p| <= 0.022, below e4m3's normal
range), so the fp8 factors are split: weights are scaled x8 and the
fp8 image holds x/8 (both comfortably normal); the matmul's product is
then exact-scale. Measured accuracy: ~5e-3 max-rel vs the fp32
reference (gate: 2e-2). The center tap stays fp16.

Matmuls write full 116-wide padded rows (464 PSUM columns per 4-row
block, halo included) so every rhs is a single flat contiguous run; the
ACT copy extracts the 112-wide interior and downcasts to fp16, halving
output DMA bytes. Image loads: fp16 chunks on the SP ring, fp8 chunks
on the DVE ring (idle), outputs on the ACT ring.
"""

import os
import sys

# The grading/bench environment may pin JAX_PLATFORMS=cpu for the jax
# reference; this kernel needs the axon/neuron PJRT backend.
if os.environ.get("JAX_PLATFORMS") == "cpu":
    del os.environ["JAX_PLATFORMS"]

for _p in ("/opt/trn_rl_repo",):
    if os.path.isdir(_p) and _p not in sys.path:
        sys.path.append(_p)

import ml_dtypes
import numpy as np

import concourse.bass as bass
import concourse.mybir as mybir
from concourse import bacc
from concourse.ap import AP
from concourse.bass_utils import run_bass_kernel_spmd
from concourse.tile import TileContext

O = 128
I = 128
B = 32
H = 112
W = 112
NCORES = 8
BPC = B // NCORES  # images per core
HP = H + 2  # padded rows (interior at row 1)
WP = W + 4  # padded cols, stride 116 (interior at col 2)
C0 = 2  # interior column offset
G8 = 2  # fp8 image lead guard elems (reads at col -1 of row 0)
SZ16 = HP * WP  # fp16 image flat size
SZ8 = HP * WP + 4  # fp8 image flat size incl guards
RB = 4  # output rows per PSUM block (464 cols incl halo <= 512)
NBLK = H // RB  # 28
NPS = RB * WP  # 464 psum columns per block
NOUT = RB * W  # 448 interior outputs per block
WSCALE = 8.0  # fp8 split: weights x8, image x/8
F32 = mybir.dt.float32
F16 = mybir.dt.float16
F8 = mybir.dt.float8e4
E4M3 = ml_dtypes.float8_e4m3fn

# image-load chunks in padded-row units (contiguous). Image 0 is on the
# critical path: its chunks are spread across the SP/ACT/GpSimd rings in
# consumption order so rows arrive just ahead of the PE. Prefetched
# images use coarse chunks (fewer descriptors).
CHUNKS = [(0, 18), (18, 58), (58, HP)]
# output-DMA group sizes (blocks per out tile). Early images use big,
# late groups so output bursts don't steal DMA bandwidth from the
# input stream while it ramps; the last image tapers so the final
# copy+DMA chain after the last matmul is short.
OUT_GROUPS = [8, 8, 8, 4]
OUT_GROUPS_LAST = [8, 8, 8, 2, 1, 1]

# periphery tap pairs for the 4 DoubleRow matmuls
PAIRS = [
    ((-1, -1), (-1, 1)),
    ((-1, 0), (1, 0)),
    ((0, -1), (0, 1)),
    ((1, -1), (1, 1)),
]


def synth_weights(core, periphery, threshold, scale):
    """Host-side weight synthesis.

    Returns (c16, w8):
      c16 (I, O) fp16 lhsT for the center tap (weights c).
      w8 (I, 4*2*O) e4m3 lhsT for the 4 DoubleRow pairs:
        w8[i, ((q*2)+t)*O + o] = 8 * cg[o,i] * p_tap(PAIRS[q][t]).
    """
    c = np.asarray(core, np.float64)[:, :, 0, 0]  # (O, I)
    thr = np.asarray(threshold, np.float64)
    s = float(np.asarray(scale, np.float64)[0])
    p = np.asarray(periphery, np.float64)
    gate = 1.0 / (1.0 + np.exp(-s * (np.abs(c) - thr[:, None])))
    cg = c * gate
    pf = np.concatenate([p[:4], [1.0], p[4:]])  # (9,) row-major taps

    c16 = np.ascontiguousarray(c.T.astype(np.float16))  # (I, O)

    w8 = np.empty((I, 4, 2, O), np.float64)
    for q, pair in enumerate(PAIRS):
        for t, (dh, dw) in enumerate(pair):
            w8[:, q, t, :] = (cg * pf[(dh + 1) * 3 + (dw + 1)] * WSCALE).T
    w8 = np.ascontiguousarray(w8.reshape(I, 4 * 2 * O).astype(E4M3))
    return c16, w8


def _pair_rhs(img8, h0, q):
    """Interleaved rhs AP for DoubleRow pair q at output block h0:
    [partition][plane=2 (stride delta)][pixel=464 (stride 1)]."""
    (dha, dwa), (dhb, dwb) = PAIRS[q]
    start = G8 + (h0 + 1 + dha) * WP + dwa
    delta = (dhb - dha) * WP + (dwb - dwa)
    base = img8[:]
    return AP(base.tensor, base.offset + start, [list(base.ap[0]), [delta, 2], [1, NPS]])


def build_nc():
    nc = bacc.Bacc(None)
    x16_d = nc.dram_tensor("x16", [BPC, I, SZ16], F16, kind="ExternalInput")
    x8_d = nc.dram_tensor("x8", [BPC, I, SZ8], F8, kind="ExternalInput")
    c16_d = nc.dram_tensor("c16", [I, O], F16, kind="ExternalInput")
    w8_d = nc.dram_tensor("w8", [I, 4 * 2 * O], F8, kind="ExternalInput")

    y_d = nc.dram_tensor("y", [BPC, O, H * W], F16, kind="ExternalOutput")

    with TileContext(nc) as tc, tc.tile_pool(name="persist", bufs=1) as persist:
        c16 = persist.tile([I, O], F16, name="c16", tag="c16")
        w8 = persist.tile([I, 4 * 2 * O], F8, name="w8", tag="w8")
        nc.sync.dma_start(out=c16[:], in_=c16_d[:])
        nc.sync.dma_start(out=w8[:], in_=w8_d[:])

        # HAM warmup: the PE clock gate sits at 1.2 GHz until ~3.4us of
        # sustained matmul activity. A dependency-free burst right after
        # engine boot flips it to 2.4 GHz before the first real matmul.
        warm = persist.tile([128, 640], F16, name="warm", tag="warm")
        nc.vector.memset(warm[:], 0.0)
        w8v = w8.rearrange("p (q t o) -> p q t o", q=4, t=2)

        imgs16 = []
        imgs8 = []
        for ib in range(3):
            imgs16.append(
                persist.tile([128, SZ16], F16, name=f"i16_{ib}", tag=f"i16_{ib}")
            )
            imgs8.append(
                persist.tile([128, SZ8], F8, name=f"i8_{ib}", tag=f"i8_{ib}")
            )

        def _dma16(eng, b, r0, r1):
            eng.dma_start(
                out=imgs16[b % 3][:, r0 * WP : r1 * WP],
                in_=x16_d[b][:, r0 * WP : r1 * WP],
            )

        def _dma8(eng, b):
            # one DMA per fp8 image: the hand-built DoubleRow rhs APs fall
            # back to whole-tile dependency tracking, so every pair matmul
            # of image b waits for image b's LAST fp8 byte anyway — make
            # that a single early transfer.
            eng.dma_start(out=imgs8[b % 3][:], in_=x8_d[b][:])

        def load_image(b):
            _dma8(nc.gpsimd, b)
            for r0, r1 in CHUNKS:
                _dma16(nc.sync, b, r0, r1)

        def load_image0():
            # img0's fp8 is the first SP-ring item: block 0's pair
            # matmuls stall on its completion (~5us of aggregate DMA).
            _dma8(nc.sync, 0)
            # ACT ring: fp16 rows in consumption order, fine-grained.
            for r0, r1 in ((0, 6), (6, 18), (18, 42)):
                _dma16(nc.scalar, 0, r0, r1)
            _dma16(nc.sync, 0, 42, 74)
            _dma16(nc.sync, 0, 74, HP)

        with (
            tc.tile_pool(name="psum", bufs=8, space="PSUM") as psum_pool,
            tc.tile_pool(name="outp", bufs=3) as out_pool,
        ):
            # per-image block index -> (group start block, group size)
            group_maps = []
            for groups in (OUT_GROUPS, OUT_GROUPS_LAST):
                gm = {}
                blk0 = 0
                for ng in groups:
                    for j in range(ng):
                        gm[blk0 + j] = (blk0, ng)
                    blk0 += ng
                assert blk0 == NBLK
                group_maps.append(gm)

            for k in range(10):
                pw = psum_pool.tile([128, 512], F32, name="pw", tag="ps")
                nc.tensor.matmul(
                    out=pw[:],
                    lhsT=warm[:, 0:128],
                    rhs=warm[:, 128:640],
                    start=True,
                    stop=True,
                )
            load_image0()
            load_image(1)
            for b in range(BPC):
                if b + 2 < BPC:
                    load_image(b + 2)
                i16 = imgs16[b % 3]
                i8 = imgs8[b % 3]
                group_of = group_maps[1 if b == BPC - 1 else 0]
                yflat = y_d[b]
                ot = None
                ot3 = None
                for blk in range(NBLK):
                    g0, gsz = group_of[blk]
                    if blk == g0:
                        ot = out_pool.tile([128, gsz * NOUT], F16, name="ot", tag="ot")
                        ot3 = ot.rearrange("p (g h w) -> p g h w", h=RB, w=W)
                    h0 = blk * RB
                    ps = psum_pool.tile([128, NPS], F32, name="ps")
                    nc.tensor.matmul(
                        out=ps[:],
                        lhsT=c16[:],
                        rhs=i16[:, (h0 + 1) * WP : (h0 + 1) * WP + NPS],
                        start=True,
                        stop=False,
                    )
                    for q in range(4):
                        nc.tensor.matmul(
                            out=ps[:],
                            lhsT=w8v[:, q],
                            rhs=_pair_rhs(i8, h0, q),
                            start=False,
                            stop=(q == 3),
                            perf_mode=mybir.MatmulPerfMode.DoubleRow,
                        )
                    ps3 = ps.rearrange("p (h w) -> p h w", w=WP)
                    nc.scalar.copy(
                        out=ot3[:, blk - g0], in_=ps3[:, :, C0 : C0 + W]
                    )
                    if blk == g0 + gsz - 1:
                        nc.scalar.dma_start(
                            out=yflat[:, g0 * NOUT : (g0 + gsz) * NOUT], in_=ot[:]
                        )
    nc.finalize()
    return nc


_NC_CACHE = {}


def _get_nc():
    if "nc" not in _NC_CACHE:
        _NC_CACHE["nc"] = build_nc()
    return _NC_CACHE["nc"]


def _prep_images(x):
    """(B, I, H, W) fp32 -> fp16 padded flat + e4m3 (x/8) padded flat."""
    xp = np.zeros((B, I, HP, WP), np.float32)
    xp[:, :, 1 : 1 + H, C0 : C0 + W] = x
    x16 = xp.astype(np.float16).reshape(B, I, SZ16)
    x8 = np.zeros((B, I, SZ8), E4M3)
    x8[:, :, G8 : G8 + SZ16] = (xp / WSCALE).astype(E4M3).reshape(B, I, SZ16)
    return np.ascontiguousarray(x16), np.ascontiguousarray(x8)


def run(inputs, trace=False, **kw):
    """Run on hardware; returns (y, BassKernelResults)."""
    x = np.asarray(inputs["x"], np.float32)
    assert x.shape == (B, I, H, W), x.shape
    c16, w8 = synth_weights(
        inputs["core"], inputs["periphery"], inputs["threshold"], inputs["scale"]
    )
    x16, x8 = _prep_images(x)
    nc = _get_nc()
    in_maps = [
        {
            "x16": x16[c * BPC : (c + 1) * BPC],
            "x8": x8[c * BPC : (c + 1) * BPC],
            "c16": c16,
            "w8": w8,
        }
        for c in range(NCORES)
    ]
    res = run_bass_kernel_spmd(nc, in_maps, list(range(NCORES)), trace=trace, **kw)
    y = np.concatenate(
        [res.results[c]["y"].reshape(BPC, O, H, W) for c in range(NCORES)], axis=0
    ).astype(np.float32)
    return y, res


def kernel(**inputs) -> np.ndarray:
    y, _ = run(inputs)
    return y


# revision 19
# speedup vs baseline: 1.0531x; 1.0531x over previous
"""CTConv2d Trainium2 kernel — fp8 DoubleRow edition.

Computes y = conv2d(x, w) where w (O,I,3,3) is synthesized on host from
core/periphery/threshold/scale, and the conv runs on 8 NeuronCores,
data-parallel over batch (32 images -> 4 per core).

Per 4-row output block the PE runs FIVE matmuls instead of the
baseline's seven:
  - 1 fp16 matmul for the dominant center tap (weights c), and
  - 4 fp8e4m3 DoubleRow matmuls, each computing TWO periphery taps at
    once: the PE's double-fp8 mode contracts two (weight, pixel) pairs
    per cell per cycle. The two taps of a pair read the SAME padded fp8
    image at two different (dh,dw) shifts via a 3D access pattern
    [partition][plane=2, stride=delta][pixel=464, stride 1] — zero data
    movement, no interleaving copies, and the Vector engine is entirely
    idle (the baseline spent ~115us of DVE time on tap pre-combining).

Periphery weights are tiny (|cg·p| <= 0.022, below e4m3's normal
range), so the fp8 factors are split: weights are scaled x8 and the
fp8 image holds x/8 (both comfortably normal); the matmul's product is
then exact-scale. Measured accuracy: ~5e-3 max-rel vs the fp32
reference (gate: 2e-2). The center tap stays fp16.

Matmuls write full 116-wide padded rows (464 PSUM columns per 4-row
block, halo included) so every rhs is a single flat contiguous run; the
ACT copy extracts the 112-wide interior and downcasts to fp16, halving
output DMA bytes. Image loads: fp16 chunks on the SP ring, fp8 chunks
on the DVE ring (idle), outputs on the ACT ring.
"""

import os
import sys

# The grading/bench environment may pin JAX_PLATFORMS=cpu for the jax
# reference; this kernel needs the axon/neuron PJRT backend.
if os.environ.get("JAX_PLATFORMS") == "cpu":
    del os.environ["JAX_PLATFORMS"]

for _p in ("/opt/trn_rl_repo",):
    if os.path.isdir(_p) and _p not in sys.path:
        sys.path.append(_p)

import ml_dtypes
import numpy as np

import concourse.bass as bass
import concourse.mybir as mybir
from concourse import bacc
from concourse.ap import AP
from concourse.bass_utils import run_bass_kernel_spmd
from concourse.tile import TileContext

O = 128
I = 128
B = 32
H = 112
W = 112
NCORES = 8
BPC = B // NCORES  # images per core
HP = H + 2  # padded rows (interior at row 1)
WP = W + 4  # padded cols, stride 116 (interior at col 2)
C0 = 2  # interior column offset
G8 = 2  # fp8 image lead guard elems (reads at col -1 of row 0)
SZ16 = HP * WP  # fp16 image flat size
SZ8 = HP * WP + 4  # fp8 image flat size incl guards
RB = 4  # output rows per PSUM block (464 cols incl halo <= 512)
NBLK = H // RB  # 28
NPS = RB * WP  # 464 psum columns per block
NOUT = RB * W  # 448 interior outputs per block
WSCALE = 8.0  # fp8 split: weights x8, image x/8
F32 = mybir.dt.float32
F16 = mybir.dt.float16
F8 = mybir.dt.float8e4
E4M3 = ml_dtypes.float8_e4m3fn

# image-load chunks in padded-row units (contiguous). Image 0 is on the
# critical path: its chunks are spread across the SP/ACT/GpSimd rings in
# consumption order so rows arrive just ahead of the PE. Prefetched
# images use coarse chunks (fewer descriptors).
CHUNKS = [(0, 18), (18, 58), (58, HP)]
# output-DMA group sizes (blocks per out tile). Early images use big,
# late groups so output bursts don't steal DMA bandwidth from the
# input stream while it ramps; the last image tapers so the final
# copy+DMA chain after the last matmul is short.
OUT_GROUPS = [8, 8, 8, 4]
OUT_GROUPS_LAST = [8, 8, 8, 2, 1, 1]

# periphery tap pairs for the 4 DoubleRow matmuls
PAIRS = [
    ((-1, -1), (-1, 1)),
    ((-1, 0), (1, 0)),
    ((0, -1), (0, 1)),
    ((1, -1), (1, 1)),
]


def synth_weights(core, periphery, threshold, scale):
    """Host-side weight synthesis.

    Returns (c16, w8):
      c16 (I, O) fp16 lhsT for the center tap (weights c).
      w8 (I, 4*2*O) e4m3 lhsT for the 4 DoubleRow pairs:
        w8[i, ((q*2)+t)*O + o] = 8 * cg[o,i] * p_tap(PAIRS[q][t]).
    """
    c = np.asarray(core, np.float64)[:, :, 0, 0]  # (O, I)
    thr = np.asarray(threshold, np.float64)
    s = float(np.asarray(scale, np.float64)[0])
    p = np.asarray(periphery, np.float64)
    gate = 1.0 / (1.0 + np.exp(-s * (np.abs(c) - thr[:, None])))
    cg = c * gate
    pf = np.concatenate([p[:4], [1.0], p[4:]])  # (9,) row-major taps

    c16 = np.ascontiguousarray(c.T.astype(np.float16))  # (I, O)

    w8 = np.empty((I, 4, 2, O), np.float64)
    for q, pair in enumerate(PAIRS):
        for t, (dh, dw) in enumerate(pair):
            w8[:, q, t, :] = (cg * pf[(dh + 1) * 3 + (dw + 1)] * WSCALE).T
    w8 = np.ascontiguousarray(w8.reshape(I, 4 * 2 * O).astype(E4M3))
    return c16, w8


def _pair_rhs(img8, h0, q):
    """Interleaved rhs AP for DoubleRow pair q at output block h0:
    [partition][plane=2 (stride delta)][pixel=464 (stride 1)]."""
    (dha, dwa), (dhb, dwb) = PAIRS[q]
    start = G8 + (h0 + 1 + dha) * WP + dwa
    delta = (dhb - dha) * WP + (dwb - dwa)
    base = img8[:]
    return AP(base.tensor, base.offset + start, [list(base.ap[0]), [delta, 2], [1, NPS]])


def build_nc():
    nc = bacc.Bacc(None)
    x16_d = nc.dram_tensor("x16", [BPC, I, SZ16], F16, kind="ExternalInput")
    x8_d = nc.dram_tensor("x8", [BPC, I, SZ8], F8, kind="ExternalInput")
    c16_d = nc.dram_tensor("c16", [I, O], F16, kind="ExternalInput")
    w8_d = nc.dram_tensor("w8", [I, 4 * 2 * O], F8, kind="ExternalInput")

    y_d = nc.dram_tensor("y", [BPC, O, H * W], F16, kind="ExternalOutput")

    with TileContext(nc) as tc, tc.tile_pool(name="persist", bufs=1) as persist:
        c16 = persist.tile([I, O], F16, name="c16", tag="c16")
        w8 = persist.tile([I, 4 * 2 * O], F8, name="w8", tag="w8")
        nc.sync.dma_start(out=c16[:], in_=c16_d[:])
        nc.sync.dma_start(out=w8[:], in_=w8_d[:])

        # HAM warmup: the PE clock gate sits at 1.2 GHz until ~3.4us of
        # sustained matmul activity. A dependency-free burst right after
        # engine boot flips it to 2.4 GHz before the first real matmul.
        warm = persist.tile([128, 640], F16, name="warm", tag="warm")
        nc.vector.memset(warm[:], 0.0)
        w8v = w8.rearrange("p (q t o) -> p q t o", q=4, t=2)

        imgs16 = []
        imgs8 = []
        for ib in range(3):
            imgs16.append(
                persist.tile([128, SZ16], F16, name=f"i16_{ib}", tag=f"i16_{ib}")
            )
            imgs8.append(
                persist.tile([128, SZ8], F8, name=f"i8_{ib}", tag=f"i8_{ib}")
            )

        def _dma16(eng, b, r0, r1):
            eng.dma_start(
                out=imgs16[b % 3][:, r0 * WP : r1 * WP],
                in_=x16_d[b][:, r0 * WP : r1 * WP],
            )

        def _dma8(eng, b):
            # one DMA per fp8 image: the hand-built DoubleRow rhs APs fall
            # back to whole-tile dependency tracking, so every pair matmul
            # of image b waits for image b's LAST fp8 byte anyway — make
            # that a single early transfer.
            eng.dma_start(out=imgs8[b % 3][:], in_=x8_d[b][:])

        def load_image(b):
            # Images 1-2 ride the SP ring FIFO behind image 0's transfers
            # so their descriptors can't steal queue bandwidth from the
            # startup-critical image 0 stream. Image 3 goes on the GpSimd
            # ring: its buffer-reuse WAR dependency (buf 0, freed when
            # image 0's last matmul retires) paces it automatically.
            eng = nc.gpsimd if b >= 3 else nc.sync
            _dma8(eng, b)
            for r0, r1 in CHUNKS:
                _dma16(eng, b, r0, r1)

        def load_image0():
            # img0's fp8 is the first bulk SP-ring item: block 0's pair
            # matmuls stall on its completion (~5us of aggregate DMA).
            _dma8(nc.sync, 0)
            # ACT ring in parallel: fp16 rows in consumption order.
            for r0, r1 in ((0, 6), (6, 18), (18, 42)):
                _dma16(nc.scalar, 0, r0, r1)
            _dma16(nc.sync, 0, 42, 74)
            _dma16(nc.sync, 0, 74, HP)

        with (
            tc.tile_pool(name="psum", bufs=8, space="PSUM") as psum_pool,
            tc.tile_pool(name="outp", bufs=3) as out_pool,
        ):
            # per-image block index -> (group start block, group size)
            group_maps = []
            for groups in (OUT_GROUPS, OUT_GROUPS_LAST):
                gm = {}
                blk0 = 0
                for ng in groups:
                    for j in range(ng):
                        gm[blk0 + j] = (blk0, ng)
                    blk0 += ng
                assert blk0 == NBLK
                group_maps.append(gm)

            for k in range(10):
                pw = psum_pool.tile([128, 512], F32, name="pw", tag="ps")
                nc.tensor.matmul(
                    out=pw[:],
                    lhsT=warm[:, 0:128],
                    rhs=warm[:, 128:640],
                    start=True,
                    stop=True,
                )
            load_image0()
            load_image(1)
            for b in range(BPC):
                if b + 2 < BPC:
                    load_image(b + 2)
                i16 = imgs16[b % 3]
                i8 = imgs8[b % 3]
                group_of = group_maps[1 if b == BPC - 1 else 0]
                yflat = y_d[b]
                ot = None
                ot3 = None
                for blk in range(NBLK):
                    g0, gsz = group_of[blk]
                    if blk == g0:
                        ot = out_pool.tile([128, gsz * NOUT], F16, name="ot", tag="ot")
                        ot3 = ot.rearrange("p (g h w) -> p g h w", h=RB, w=W)
                    h0 = blk * RB
                    ps = psum_pool.tile([128, NPS], F32, name="ps")
                    nc.tensor.matmul(
                        out=ps[:],
                        lhsT=c16[:],
                        rhs=i16[:, (h0 + 1) * WP : (h0 + 1) * WP + NPS],
                        start=True,
                        stop=False,
                    )
                    for q in range(4):
                        nc.tensor.matmul(
                            out=ps[:],
                            lhsT=w8v[:, q],
                            rhs=_pair_rhs(i8, h0, q),
                            start=False,
                            stop=(q == 3),
                            perf_mode=mybir.MatmulPerfMode.DoubleRow,
                        )
                    ps3 = ps.rearrange("p (h w) -> p h w", w=WP)
                    nc.scalar.copy(
                        out=ot3[:, blk - g0], in_=ps3[:, :, C0 : C0 + W]
                    )
                    if blk == g0 + gsz - 1:
                        nc.scalar.dma_start(
                            out=yflat[:, g0 * NOUT : (g0 + gsz) * NOUT], in_=ot[:]
                        )
    nc.finalize()
    return nc


_NC_CACHE = {}


def _get_nc():
    if "nc" not in _NC_CACHE:
        _NC_CACHE["nc"] = build_nc()
    return _NC_CACHE["nc"]


def _prep_images(x):
    """(B, I, H, W) fp32 -> fp16 padded flat + e4m3 (x/8) padded flat."""
    xp = np.zeros((B, I, HP, WP), np.float32)
    xp[:, :, 1 : 1 + H, C0 : C0 + W] = x
    x16 = xp.astype(np.float16).reshape(B, I, SZ16)
    x8 = np.zeros((B, I, SZ8), E4M3)
    x8[:, :, G8 : G8 + SZ16] = (xp / WSCALE).astype(E4M3).reshape(B, I, SZ16)
    return np.ascontiguousarray(x16), np.ascontiguousarray(x8)


def run(inputs, trace=False, **kw):
    """Run on hardware; returns (y, BassKernelResults)."""
    x = np.asarray(inputs["x"], np.float32)
    assert x.shape == (B, I, H, W), x.shape
    c16, w8 = synth_weights(
        inputs["core"], inputs["periphery"], inputs["threshold"], inputs["scale"]
    )
    x16, x8 = _prep_images(x)
    nc = _get_nc()
    in_maps = [
        {
            "x16": x16[c * BPC : (c + 1) * BPC],
            "x8": x8[c * BPC : (c + 1) * BPC],
            "c16": c16,
            "w8": w8,
        }
        for c in range(NCORES)
    ]
    res = run_bass_kernel_spmd(nc, in_maps, list(range(NCORES)), trace=trace, **kw)
    y = np.concatenate(
        [res.results[c]["y"].reshape(BPC, O, H, W) for c in range(NCORES)], axis=0
    ).astype(np.float32)
    return y, res


def kernel(**inputs) -> np.ndarray:
    y, _ = run(inputs)
    return y


# revision 28
# speedup vs baseline: 1.0909x; 1.0358x over previous
"""CTConv2d Trainium2 kernel — fp8 DoubleRow edition.

Computes y = conv2d(x, w) where w (O,I,3,3) is synthesized on host from
core/periphery/threshold/scale, and the conv runs on 8 NeuronCores,
data-parallel over batch (32 images -> 4 per core).

Per 4-row output block the PE runs FIVE matmuls instead of the
baseline's seven:
  - 1 fp16 matmul for the dominant center tap (weights c), and
  - 4 fp8e4m3 DoubleRow matmuls, each computing TWO periphery taps at
    once: the PE's double-fp8 mode contracts two (weight, pixel) pairs
    per cell per cycle. The two taps of a pair read the SAME padded fp8
    image at two different (dh,dw) shifts via a 3D access pattern
    [partition][plane=2, stride=delta][pixel=464, stride 1] — zero data
    movement, no interleaving copies, and the Vector engine is entirely
    idle (the baseline spent ~115us of DVE time on tap pre-combining).

Periphery weights are tiny (|cg·p| <= 0.022, below e4m3's normal
range), so the fp8 factors are split: weights are scaled x8 and the
fp8 image holds x/8 (both comfortably normal); the matmul's product is
then exact-scale. Measured accuracy: ~5e-3 max-rel vs the fp32
reference (gate: 2e-2). The center tap stays fp16.

Matmuls write full 116-wide padded rows (464 PSUM columns per 4-row
block, halo included) so every rhs is a single flat contiguous run; the
ACT copy extracts the 112-wide interior and downcasts to fp16, halving
output DMA bytes. Image loads: fp16 chunks on the SP ring, fp8 chunks
on the DVE ring (idle), outputs on the ACT ring.
"""

import os
import sys

# The grading/bench environment may pin JAX_PLATFORMS=cpu for the jax
# reference; this kernel needs the axon/neuron PJRT backend.
if os.environ.get("JAX_PLATFORMS") == "cpu":
    del os.environ["JAX_PLATFORMS"]

for _p in ("/opt/trn_rl_repo",):
    if os.path.isdir(_p) and _p not in sys.path:
        sys.path.append(_p)

import ml_dtypes
import numpy as np

import concourse.bass as bass
import concourse.mybir as mybir
from concourse import bacc
from concourse.ap import AP
from concourse.bass_utils import run_bass_kernel_spmd
from concourse.tile import TileContext

O = 128
I = 128
B = 32
H = 112
W = 112
NCORES = 8
BPC = B // NCORES  # images per core
HP = H + 2  # padded rows (interior at row 1)
WP = W + 4  # padded cols, stride 116 (interior at col 2)
C0 = 2  # interior column offset
G8 = 2  # fp8 image lead guard elems (reads at col -1 of row 0)
SZ16 = HP * WP  # fp16 image flat size
# The hand-built DoubleRow rhs APs fall back to whole-tile dependency
# tracking, so a pair matmul waits for its fp8 tile's LAST byte. Each
# fp8 image is therefore split into two overlapping half-tiles (padded
# rows [0,62) and [56,114)) with independent DMAs: blocks 0-13 only
# gate on the first ~0.9MB, halving the startup stall.
R8B = 56  # first padded row of the second fp8 half-tile
SPLIT_BLK = 14  # blocks >= this read the second half-tile
SZ8A = G8 + 62 * WP + 2
SZ8B = G8 + (HP - R8B) * WP + 2
RB = 4  # output rows per PSUM block (464 cols incl halo <= 512)
NBLK = H // RB  # 28
NPS = RB * WP  # 464 psum columns per block
NOUT = RB * W  # 448 interior outputs per block
WSCALE = 8.0  # fp8 split: weights x8, image x/8
F32 = mybir.dt.float32
F16 = mybir.dt.float16
F8 = mybir.dt.float8e4
E4M3 = ml_dtypes.float8_e4m3fn

# image-load chunks in padded-row units (contiguous). Image 0 is on the
# critical path: its chunks are spread across the SP/ACT/GpSimd rings in
# consumption order so rows arrive just ahead of the PE. Prefetched
# images use coarse chunks (fewer descriptors).
CHUNKS = [(0, 18), (18, 58), (58, HP)]
# output-DMA group sizes (blocks per out tile). Early images use big,
# late groups so output bursts don't steal DMA bandwidth from the
# input stream while it ramps; the last image tapers so the final
# copy+DMA chain after the last matmul is short.
OUT_GROUPS = [8, 8, 8, 4]
OUT_GROUPS_LAST = [8, 8, 8, 2, 1, 1]

# periphery tap pairs for the 4 DoubleRow matmuls
PAIRS = [
    ((-1, -1), (-1, 1)),
    ((-1, 0), (1, 0)),
    ((0, -1), (0, 1)),
    ((1, -1), (1, 1)),
]


def synth_weights(core, periphery, threshold, scale):
    """Host-side weight synthesis.

    Returns (c16, w8):
      c16 (I, O) fp16 lhsT for the center tap (weights c).
      w8 (I, 4*2*O) e4m3 lhsT for the 4 DoubleRow pairs:
        w8[i, ((q*2)+t)*O + o] = 8 * cg[o,i] * p_tap(PAIRS[q][t]).
    """
    c = np.asarray(core, np.float64)[:, :, 0, 0]  # (O, I)
    thr = np.asarray(threshold, np.float64)
    s = float(np.asarray(scale, np.float64)[0])
    p = np.asarray(periphery, np.float64)
    gate = 1.0 / (1.0 + np.exp(-s * (np.abs(c) - thr[:, None])))
    cg = c * gate
    pf = np.concatenate([p[:4], [1.0], p[4:]])  # (9,) row-major taps

    c16 = np.ascontiguousarray(c.T.astype(np.float16))  # (I, O)

    w8 = np.empty((I, 4, 2, O), np.float64)
    for q, pair in enumerate(PAIRS):
        for t, (dh, dw) in enumerate(pair):
            w8[:, q, t, :] = (cg * pf[(dh + 1) * 3 + (dw + 1)] * WSCALE).T
    w8 = np.ascontiguousarray(w8.reshape(I, 4 * 2 * O).astype(E4M3))
    return c16, w8


def _pair_rhs(img8, h0, q, row0):
    """Interleaved rhs AP for DoubleRow pair q at output block h0:
    [partition][plane=2 (stride delta)][pixel=464 (stride 1)].
    row0 is the padded row the tile's interior starts at."""
    (dha, dwa), (dhb, dwb) = PAIRS[q]
    start = G8 + (h0 + 1 + dha - row0) * WP + dwa
    delta = (dhb - dha) * WP + (dwb - dwa)
    base = img8[:]
    return AP(base.tensor, base.offset + start, [list(base.ap[0]), [delta, 2], [1, NPS]])


def build_nc():
    nc = bacc.Bacc(None)
    x16_d = nc.dram_tensor("x16", [BPC, I, SZ16], F16, kind="ExternalInput")
    x8a_d = nc.dram_tensor("x8a", [BPC, I, SZ8A], F8, kind="ExternalInput")
    x8b_d = nc.dram_tensor("x8b", [BPC, I, SZ8B], F8, kind="ExternalInput")
    c16_d = nc.dram_tensor("c16", [I, O], F16, kind="ExternalInput")
    w8_d = nc.dram_tensor("w8", [I, 4 * 2 * O], F8, kind="ExternalInput")

    y_d = nc.dram_tensor("y", [BPC, O, H * W], F16, kind="ExternalOutput")

    with TileContext(nc) as tc, tc.tile_pool(name="persist", bufs=1) as persist:
        c16 = persist.tile([I, O], F16, name="c16", tag="c16")
        w8 = persist.tile([I, 4 * 2 * O], F8, name="w8", tag="w8")
        nc.sync.dma_start(out=c16[:], in_=c16_d[:])
        nc.sync.dma_start(out=w8[:], in_=w8_d[:])

        # HAM warmup: the PE clock gate sits at 1.2 GHz until ~3.4us of
        # sustained matmul activity. A dependency-free burst right after
        # engine boot flips it to 2.4 GHz before the first real matmul.
        warm = persist.tile([128, 640], F16, name="warm", tag="warm")
        nc.vector.memset(warm[:], 0.0)
        w8v = w8.rearrange("p (q t o) -> p q t o", q=4, t=2)

        imgs16 = []
        imgs8a = []
        imgs8b = []
        for ib in range(3):
            imgs16.append(
                persist.tile([128, SZ16], F16, name=f"i16_{ib}", tag=f"i16_{ib}")
            )
            imgs8a.append(
                persist.tile([128, SZ8A], F8, name=f"i8a_{ib}", tag=f"i8a_{ib}")
            )
            imgs8b.append(
                persist.tile([128, SZ8B], F8, name=f"i8b_{ib}", tag=f"i8b_{ib}")
            )

        def _dma16(eng, b, r0, r1):
            eng.dma_start(
                out=imgs16[b % 3][:, r0 * WP : r1 * WP],
                in_=x16_d[b][:, r0 * WP : r1 * WP],
            )

        def _dma8a(eng, b):
            eng.dma_start(out=imgs8a[b % 3][:], in_=x8a_d[b][:])

        def _dma8b(eng, b):
            eng.dma_start(out=imgs8b[b % 3][:], in_=x8b_d[b][:])

        def load_image(b):
            # Images 1-2 ride the SP ring FIFO behind image 0's transfers
            # so their descriptors can't steal queue bandwidth from the
            # startup-critical image 0 stream. Image 3 goes on the GpSimd
            # ring: its buffer-reuse WAR dependency (buf 0, freed when
            # image 0's last matmul retires) paces it automatically.
            eng = nc.gpsimd if b >= 3 else nc.sync
            _dma8a(eng, b)
            _dma8b(eng, b)
            for r0, r1 in CHUNKS:
                _dma16(eng, b, r0, r1)

        def load_image0():
            # img0's first fp8 half is the first bulk SP-ring item:
            # block 0's pair matmuls stall on its completion.
            _dma8a(nc.sync, 0)
            # ACT ring in parallel: fp16 rows in consumption order.
            for r0, r1 in ((0, 6), (6, 18), (18, 42)):
                _dma16(nc.scalar, 0, r0, r1)
            _dma16(nc.sync, 0, 42, 74)
            _dma8b(nc.sync, 0)
            _dma16(nc.sync, 0, 74, HP)

        with (
            tc.tile_pool(name="psum", bufs=8, space="PSUM") as psum_pool,
            tc.tile_pool(name="outp", bufs=3) as out_pool,
        ):
            # per-image block index -> (group start block, group size)
            group_maps = []
            for groups in (OUT_GROUPS, OUT_GROUPS_LAST):
                gm = {}
                blk0 = 0
                for ng in groups:
                    for j in range(ng):
                        gm[blk0 + j] = (blk0, ng)
                    blk0 += ng
                assert blk0 == NBLK
                group_maps.append(gm)

            for k in range(10):
                pw = psum_pool.tile([128, 512], F32, name="pw", tag="ps")
                nc.tensor.matmul(
                    out=pw[:],
                    lhsT=warm[:, 0:128],
                    rhs=warm[:, 128:640],
                    start=True,
                    stop=True,
                )
            load_image0()
            load_image(1)
            for b in range(BPC):
                if b + 2 < BPC:
                    load_image(b + 2)
                i16 = imgs16[b % 3]
                i8a = imgs8a[b % 3]
                i8b = imgs8b[b % 3]
                group_of = group_maps[1 if b == BPC - 1 else 0]
                yflat = y_d[b]
                ot = None
                ot3 = None
                for blk in range(NBLK):
                    g0, gsz = group_of[blk]
                    if blk == g0:
                        ot = out_pool.tile([128, gsz * NOUT], F16, name="ot", tag="ot")
                        ot3 = ot.rearrange("p (g h w) -> p g h w", h=RB, w=W)
                    h0 = blk * RB
                    ps = psum_pool.tile([128, NPS], F32, name="ps")
                    nc.tensor.matmul(
                        out=ps[:],
                        lhsT=c16[:],
                        rhs=i16[:, (h0 + 1) * WP : (h0 + 1) * WP + NPS],
                        start=True,
                        stop=False,
                    )
                    i8, row0 = (i8a, 0) if blk < SPLIT_BLK else (i8b, R8B)
                    for q in range(4):
                        nc.tensor.matmul(
                            out=ps[:],
                            lhsT=w8v[:, q],
                            rhs=_pair_rhs(i8, h0, q, row0),
                            start=False,
                            stop=(q == 3),
                            perf_mode=mybir.MatmulPerfMode.DoubleRow,
                        )
                    ps3 = ps.rearrange("p (h w) -> p h w", w=WP)
                    nc.scalar.copy(
                        out=ot3[:, blk - g0], in_=ps3[:, :, C0 : C0 + W]
                    )
                    if blk == g0 + gsz - 1:
                        nc.scalar.dma_start(
                            out=yflat[:, g0 * NOUT : (g0 + gsz) * NOUT], in_=ot[:]
                        )
    nc.finalize()
    return nc


_NC_CACHE = {}


def _get_nc():
    if "nc" not in _NC_CACHE:
        _NC_CACHE["nc"] = build_nc()
    return _NC_CACHE["nc"]


def _prep_images(x):
    """(B, I, H, W) fp32 -> fp16 padded flat + two e4m3 (x/8) half-tiles."""
    xp = np.zeros((B, I, HP, WP), np.float32)
    xp[:, :, 1 : 1 + H, C0 : C0 + W] = x
    x16 = xp.astype(np.float16).reshape(B, I, SZ16)
    x8 = (xp / WSCALE).astype(E4M3)
    x8a = np.zeros((B, I, SZ8A), E4M3)
    x8a[:, :, G8 : G8 + 62 * WP] = x8[:, :, :62].reshape(B, I, 62 * WP)
    x8b = np.zeros((B, I, SZ8B), E4M3)
    x8b[:, :, G8 : G8 + (HP - R8B) * WP] = x8[:, :, R8B:].reshape(
        B, I, (HP - R8B) * WP
    )
    return np.ascontiguousarray(x16), np.ascontiguousarray(x8a), np.ascontiguousarray(x8b)


def run(inputs, trace=False, **kw):
    """Run on hardware; returns (y, BassKernelResults)."""
    x = np.asarray(inputs["x"], np.float32)
    assert x.shape == (B, I, H, W), x.shape
    c16, w8 = synth_weights(
        inputs["core"], inputs["periphery"], inputs["threshold"], inputs["scale"]
    )
    x16, x8a, x8b = _prep_images(x)
    nc = _get_nc()
    in_maps = [
        {
            "x16": x16[c * BPC : (c + 1) * BPC],
            "x8a": x8a[c * BPC : (c + 1) * BPC],
            "x8b": x8b[c * BPC : (c + 1) * BPC],
            "c16": c16,
            "w8": w8,
        }
        for c in range(NCORES)
    ]
    res = run_bass_kernel_spmd(nc, in_maps, list(range(NCORES)), trace=trace, **kw)
    y = np.concatenate(
        [res.results[c]["y"].reshape(BPC, O, H, W) for c in range(NCORES)], axis=0
    ).astype(np.float32)
    return y, res


def kernel(**inputs) -> np.ndarray:
    y, _ = run(inputs)
    return y


# revision 38
# speedup vs baseline: 1.1188x; 1.0256x over previous
"""CTConv2d Trainium2 kernel — fp8 DoubleRow edition.

Computes y = conv2d(x, w) where w (O,I,3,3) is synthesized on host from
core/periphery/threshold/scale, and the conv runs on 8 NeuronCores,
data-parallel over batch (32 images -> 4 per core).

Per 4-row output block the PE runs FIVE matmuls instead of the
baseline's seven:
  - 1 fp16 matmul for the dominant center tap (weights c), and
  - 4 fp8e4m3 DoubleRow matmuls, each computing TWO periphery taps at
    once: the PE's double-fp8 mode contracts two (weight, pixel) pairs
    per cell per cycle. The two taps of a pair read the SAME padded fp8
    image at two different (dh,dw) shifts via a 3D access pattern
    [partition][plane=2, stride=delta][pixel=464, stride 1] — zero data
    movement, no interleaving copies, and the Vector engine is entirely
    idle (the baseline spent ~115us of DVE time on tap pre-combining).

Periphery weights are tiny (|cg·p| <= 0.022, below e4m3's normal
range), so the fp8 factors are split: weights are scaled x8 and the
fp8 image holds x/8 (both comfortably normal); the matmul's product is
then exact-scale. Measured accuracy: ~5e-3 max-rel vs the fp32
reference (gate: 2e-2). The center tap stays fp16.

Matmuls write full 116-wide padded rows (464 PSUM columns per 4-row
block, halo included) so every rhs is a single flat contiguous run; the
ACT copy extracts the 112-wide interior and downcasts to fp16, halving
output DMA bytes. Image loads: fp16 chunks on the SP ring, fp8 chunks
on the DVE ring (idle), outputs on the ACT ring.
"""

import os
import sys

# The grading/bench environment may pin JAX_PLATFORMS=cpu for the jax
# reference; this kernel needs the axon/neuron PJRT backend.
if os.environ.get("JAX_PLATFORMS") == "cpu":
    del os.environ["JAX_PLATFORMS"]

for _p in ("/opt/trn_rl_repo",):
    if os.path.isdir(_p) and _p not in sys.path:
        sys.path.append(_p)

import ml_dtypes
import numpy as np

import concourse.bass as bass
import concourse.mybir as mybir
from concourse import bacc
from concourse.ap import AP
from concourse.bass_utils import run_bass_kernel_spmd
from concourse.tile import TileContext

O = 128
I = 128
B = 32
H = 112
W = 112
NCORES = 8
BPC = B // NCORES  # images per core
HP = H + 2  # padded rows (interior at row 1)
WP = W + 4  # padded cols, stride 116 (interior at col 2)
C0 = 2  # interior column offset
G8 = 2  # fp8 image lead guard elems (reads at col -1 of row 0)
SZ16 = HP * WP  # fp16 image flat size
# The hand-built DoubleRow rhs APs fall back to whole-tile dependency
# tracking, so a pair matmul waits for its fp8 tile's LAST byte. Each
# fp8 image is therefore split into two overlapping half-tiles (padded
# rows [0,62) and [56,114)) with independent DMAs: blocks 0-13 only
# gate on the first ~0.9MB, halving the startup stall.
# fp8 tile q covers padded rows [R8ROW[q], R8ROW[q]+R8LEN[q]); block
# blk (padded rows blk*4..blk*4+5) reads tile R8OF[blk].
R8ROW = (0, 14, 56)
R8LEN = (18, 48, HP - 56)
R8OF = [0] * 4 + [1] * 10 + [2] * 14
SZ8T = tuple(G8 + n * WP + 2 for n in R8LEN)
RB = 4  # output rows per PSUM block (464 cols incl halo <= 512)
NBLK = H // RB  # 28
NPS = RB * WP  # 464 psum columns per block
NOUT = RB * W  # 448 interior outputs per block
WSCALE = 8.0  # fp8 split: weights x8, image x/8
F32 = mybir.dt.float32
F16 = mybir.dt.float16
F8 = mybir.dt.float8e4
E4M3 = ml_dtypes.float8_e4m3fn

# image-load chunks in padded-row units (contiguous). Image 0 is on the
# critical path: its chunks are spread across the SP/ACT/GpSimd rings in
# consumption order so rows arrive just ahead of the PE. Prefetched
# images use coarse chunks (fewer descriptors).
CHUNKS = [(0, 18), (18, 58), (58, HP)]
# output-DMA group sizes (blocks per out tile). Early images use big,
# late groups so output bursts don't steal DMA bandwidth from the
# input stream while it ramps; the last image tapers so the final
# copy+DMA chain after the last matmul is short.
OUT_GROUPS = [8, 8, 8, 4]
OUT_GROUPS_LAST = [8, 8, 8, 2, 1, 1]

# periphery tap pairs for the 4 DoubleRow matmuls
PAIRS = [
    ((-1, -1), (-1, 1)),
    ((-1, 0), (1, 0)),
    ((0, -1), (0, 1)),
    ((1, -1), (1, 1)),
]


def synth_weights(core, periphery, threshold, scale):
    """Host-side weight synthesis.

    Returns (c16, w8):
      c16 (I, O) fp16 lhsT for the center tap (weights c).
      w8 (I, 4*2*O) e4m3 lhsT for the 4 DoubleRow pairs:
        w8[i, ((q*2)+t)*O + o] = 8 * cg[o,i] * p_tap(PAIRS[q][t]).
    """
    c = np.asarray(core, np.float64)[:, :, 0, 0]  # (O, I)
    thr = np.asarray(threshold, np.float64)
    s = float(np.asarray(scale, np.float64)[0])
    p = np.asarray(periphery, np.float64)
    gate = 1.0 / (1.0 + np.exp(-s * (np.abs(c) - thr[:, None])))
    cg = c * gate
    pf = np.concatenate([p[:4], [1.0], p[4:]])  # (9,) row-major taps

    c16 = np.ascontiguousarray(c.T.astype(np.float16))  # (I, O)

    w8 = np.empty((I, 4, 2, O), np.float64)
    for q, pair in enumerate(PAIRS):
        for t, (dh, dw) in enumerate(pair):
            w8[:, q, t, :] = (cg * pf[(dh + 1) * 3 + (dw + 1)] * WSCALE).T
    w8 = np.ascontiguousarray(w8.reshape(I, 4 * 2 * O).astype(E4M3))
    return c16, w8


def _pair_rhs(img8, h0, q, row0):
    """Interleaved rhs AP for DoubleRow pair q at output block h0:
    [partition][plane=2 (stride delta)][pixel=464 (stride 1)].
    row0 is the padded row the tile's interior starts at."""
    (dha, dwa), (dhb, dwb) = PAIRS[q]
    start = G8 + (h0 + 1 + dha - row0) * WP + dwa
    delta = (dhb - dha) * WP + (dwb - dwa)
    base = img8[:]
    return AP(base.tensor, base.offset + start, [list(base.ap[0]), [delta, 2], [1, NPS]])


def build_nc():
    nc = bacc.Bacc(None)
    x16_d = nc.dram_tensor("x16", [BPC, I, SZ16], F16, kind="ExternalInput")
    x8_ds = [
        nc.dram_tensor(f"x8{q}", [BPC, I, SZ8T[q]], F8, kind="ExternalInput")
        for q in range(3)
    ]
    c16_d = nc.dram_tensor("c16", [I, O], F16, kind="ExternalInput")
    w8_d = nc.dram_tensor("w8", [I, 4 * 2 * O], F8, kind="ExternalInput")

    y_d = nc.dram_tensor("y", [BPC, O, H * W], F16, kind="ExternalOutput")

    with TileContext(nc) as tc, tc.tile_pool(name="persist", bufs=1) as persist:
        c16 = persist.tile([I, O], F16, name="c16", tag="c16")
        w8 = persist.tile([I, 4 * 2 * O], F8, name="w8", tag="w8")
        nc.sync.dma_start(out=c16[:], in_=c16_d[:])
        nc.sync.dma_start(out=w8[:], in_=w8_d[:])

        # HAM warmup: the PE clock gate sits at 1.2 GHz until ~3.4us of
        # sustained matmul activity. A dependency-free burst right after
        # engine boot flips it to 2.4 GHz before the first real matmul.
        warm = persist.tile([128, 640], F16, name="warm", tag="warm")
        nc.vector.memset(warm[:], 0.0)
        NWARM = 8
        w8v = w8.rearrange("p (q t o) -> p q t o", q=4, t=2)

        imgs16 = []
        imgs8 = []
        for ib in range(3):
            imgs16.append(
                persist.tile([128, SZ16], F16, name=f"i16_{ib}", tag=f"i16_{ib}")
            )
            imgs8.append(
                [
                    persist.tile(
                        [128, SZ8T[q]], F8, name=f"i8{q}_{ib}", tag=f"i8{q}_{ib}"
                    )
                    for q in range(3)
                ]
            )

        def _dma16(eng, b, r0, r1):
            eng.dma_start(
                out=imgs16[b % 3][:, r0 * WP : r1 * WP],
                in_=x16_d[b][:, r0 * WP : r1 * WP],
            )

        def _dma8(eng, b, q):
            eng.dma_start(out=imgs8[b % 3][q][:], in_=x8_ds[q][b][:])

        def load_image(b):
            # Images 1-2 ride the SP ring FIFO behind image 0's transfers
            # so their descriptors can't steal queue bandwidth from the
            # startup-critical image 0 stream. Image 3 goes on the GpSimd
            # ring: its buffer-reuse WAR dependency (buf 0, freed when
            # image 0's last matmul retires) paces it automatically.
            eng = nc.gpsimd if b >= 3 else nc.sync
            for q in range(3):
                _dma8(eng, b, q)
            for r0, r1 in CHUNKS:
                _dma16(eng, b, r0, r1)

        def load_image0():
            # img0's head fp8 tile (18 rows) is the first bulk SP-ring
            # item: block 0's pair matmuls stall on its completion.
            _dma8(nc.sync, 0, 0)
            # ACT ring in parallel: fp16 rows in consumption order.
            for r0, r1 in ((0, 6), (6, 18), (18, 42)):
                _dma16(nc.scalar, 0, r0, r1)
            _dma8(nc.sync, 0, 1)
            _dma16(nc.sync, 0, 42, 74)
            _dma8(nc.sync, 0, 2)
            _dma16(nc.sync, 0, 74, HP)

        with (
            tc.tile_pool(name="psum", bufs=8, space="PSUM") as psum_pool,
            tc.tile_pool(name="outp", bufs=3) as out_pool,
        ):
            # per-image block index -> (group start block, group size)
            group_maps = []
            for groups in (OUT_GROUPS, OUT_GROUPS_LAST):
                gm = {}
                blk0 = 0
                for ng in groups:
                    for j in range(ng):
                        gm[blk0 + j] = (blk0, ng)
                    blk0 += ng
                assert blk0 == NBLK
                group_maps.append(gm)

            for k in range(NWARM):
                pw = psum_pool.tile([128, 512], F32, name="pw", tag="ps")
                nc.tensor.matmul(
                    out=pw[:],
                    lhsT=warm[:, 0:128],
                    rhs=warm[:, 128:640],
                    start=True,
                    stop=True,
                )
            load_image0()
            load_image(1)
            for b in range(BPC):
                if b + 2 < BPC:
                    load_image(b + 2)
                i16 = imgs16[b % 3]
                i8t = imgs8[b % 3]
                group_of = group_maps[1 if b == BPC - 1 else 0]
                yflat = y_d[b]
                ot = None
                ot3 = None
                for blk in range(NBLK):
                    g0, gsz = group_of[blk]
                    if blk == g0:
                        ot = out_pool.tile([128, gsz * NOUT], F16, name="ot", tag="ot")
                        ot3 = ot.rearrange("p (g h w) -> p g h w", h=RB, w=W)
                    h0 = blk * RB
                    ps = psum_pool.tile([128, NPS], F32, name="ps")
                    nc.tensor.matmul(
                        out=ps[:],
                        lhsT=c16[:],
                        rhs=i16[:, (h0 + 1) * WP : (h0 + 1) * WP + NPS],
                        start=True,
                        stop=False,
                    )
                    i8 = i8t[R8OF[blk]]
                    row0 = R8ROW[R8OF[blk]]
                    for q in range(4):
                        nc.tensor.matmul(
                            out=ps[:],
                            lhsT=w8v[:, q],
                            rhs=_pair_rhs(i8, h0, q, row0),
                            start=False,
                            stop=(q == 3),
                            perf_mode=mybir.MatmulPerfMode.DoubleRow,
                        )
                    ps3 = ps.rearrange("p (h w) -> p h w", w=WP)
                    nc.scalar.copy(
                        out=ot3[:, blk - g0], in_=ps3[:, :, C0 : C0 + W]
                    )
                    if blk == g0 + gsz - 1:
                        nc.scalar.dma_start(
                            out=yflat[:, g0 * NOUT : (g0 + gsz) * NOUT], in_=ot[:]
                        )
    nc.finalize()
    return nc


_NC_CACHE = {}


def _get_nc():
    if "nc" not in _NC_CACHE:
        _NC_CACHE["nc"] = build_nc()
    return _NC_CACHE["nc"]


def _prep_images(x):
    """(B, I, H, W) fp32 -> fp16 padded flat + three e4m3 (x/8) row tiles."""
    xp = np.zeros((B, I, HP, WP), np.float32)
    xp[:, :, 1 : 1 + H, C0 : C0 + W] = x
    x16 = np.ascontiguousarray(xp.astype(np.float16).reshape(B, I, SZ16))
    x8 = (xp / WSCALE).astype(E4M3)
    x8ts = []
    for q in range(3):
        t = np.zeros((B, I, SZ8T[q]), E4M3)
        r0, n = R8ROW[q], R8LEN[q]
        t[:, :, G8 : G8 + n * WP] = x8[:, :, r0 : r0 + n].reshape(B, I, n * WP)
        x8ts.append(np.ascontiguousarray(t))
    return x16, x8ts


def run(inputs, trace=False, **kw):
    """Run on hardware; returns (y, BassKernelResults)."""
    x = np.asarray(inputs["x"], np.float32)
    assert x.shape == (B, I, H, W), x.shape
    c16, w8 = synth_weights(
        inputs["core"], inputs["periphery"], inputs["threshold"], inputs["scale"]
    )
    x16, x8ts = _prep_images(x)
    nc = _get_nc()
    in_maps = [
        {
            "x16": x16[c * BPC : (c + 1) * BPC],
            "x80": x8ts[0][c * BPC : (c + 1) * BPC],
            "x81": x8ts[1][c * BPC : (c + 1) * BPC],
            "x82": x8ts[2][c * BPC : (c + 1) * BPC],
            "c16": c16,
            "w8": w8,
        }
        for c in range(NCORES)
    ]
    res = run_bass_kernel_spmd(nc, in_maps, list(range(NCORES)), trace=trace, **kw)
    y = np.concatenate(
        [res.results[c]["y"].reshape(BPC, O, H, W) for c in range(NCORES)], axis=0
    ).astype(np.float32)
    return y, res


def kernel(**inputs) -> np.ndarray:
    y, _ = run(inputs)
    return y
